# revision 7
# baseline (speedup 1.0000x reference)
"""GCN policy network (3-layer GCN + masked softmax + value head) on 8 TRN2 cores.

Sharding: destinations (nodes) are sharded across the 8 cores; each core owns
~12500 nodes and all edges pointing into them.  Edges are laid out host-side
into a per-core "slot grid" [128 partitions x S slots]: each partition owns a
set of destination nodes, each destination owns a run of K consecutive slots
(K = its in-degree rounded up to a multiple of 8; destinations are grouped
into tiers of equal K so segment sums become fixed-shape strided reductions).
Each slot stores the pi-space row index of its source node (or a shared zero
row for padding).

Per layer the device: (1) gathers table rows T[row] for every slot via
indirect DMA (128 rows per call, driven by a For_i loop), dumping the raw
gathered grid to DRAM; (2) reduces each tier's K-runs with log-tree strided
adds (DVE); (3) combines u with the self-loop term and applies the tiny dense
layer via pack-transpose-matmul on the TensorEngine; (4) rescales by dinv and
AllGathers the next layer's gather table.  Degrees are computed on device from
the slot indices (slot != ZROW), dinv = 1/sqrt(deg+2) on device.  The final
masked softmax + mean-pool value head run on device with one more AllGather.
"""

import sys, os
for _p in ("/opt/trn_rl_repo", "/root/.axon_site/_ro/trn_rl_repo"):
    if os.path.isdir(_p) and _p not in sys.path:
        sys.path.insert(0, _p)

import numpy as np

from concourse import bass, bacc, mybir, tile
from concourse import bass_utils

P = 128
NC = 8
F = 16           # hidden width
FIN = 3          # input feature width


# ---------------------------------------------------------------- host layout

class Layout:
    pass


def build_layout(edge_index, N):
    """Compute the per-core slot grids, node permutation and index arrays."""
    L = Layout()
    row = edge_index[0].astype(np.int64)
    col = edge_index[1].astype(np.int64)
    E = row.shape[0]
    NPC = (N + NC - 1) // NC                      # nodes per core (natural split)

    deg = np.bincount(col, minlength=N)
    K_of = np.maximum(8, 8 * np.ceil(np.maximum(deg, 1) / 8).astype(np.int64))
    tiers = sorted(set(K_of.tolist()))

    # edges sorted by destination
    order = np.argsort(col, kind="stable")
    row_s = row[order]
    starts = np.zeros(N + 1, np.int64)
    np.cumsum(deg, out=starts[1:])

    # per (core, tier) node lists and max column counts
    node_core = np.minimum(col * 0, 0)  # placeholder
    nodes_by = {}
    cols_t = {}
    for t in tiers:
        cols_t[t] = 0
    for c in range(NC):
        lo, hi = c * NPC, min((c + 1) * NPC, N)
        ids = np.arange(lo, hi)
        for t in tiers:
            sel = ids[K_of[lo:hi] == t]
            nodes_by[(c, t)] = sel
            cols_t[t] = max(cols_t[t], (len(sel) + P - 1) // P)

    # drop tiers that ended up with zero columns everywhere
    tiers = [t for t in tiers if cols_t[t] > 0]

    COLS = sum(cols_t[t] for t in tiers)
    pad_cols = (-COLS) % 8
    if pad_cols:                                  # pad with extra K=tiers[0] cols
        cols_t[tiers[0]] += pad_cols
        COLS += pad_cols
    S = sum(cols_t[t] * t for t in tiers)
    S_pad = (-S) % P
    S += S_pad

    N_g = NC * P * COLS
    ZROW = N_g

    base_col = {}
    base_slot = {}
    cc, ss = 0, 0
    for t in tiers:
        base_col[t] = cc
        base_slot[t] = ss
        cc += cols_t[t]
        ss += cols_t[t] * t

    # pi: (core, p, col) -> natural node id (or -1)
    pi = np.full((NC, P, COLS), -1, np.int64)
    for c in range(NC):
        for t in tiers:
            sel = nodes_by[(c, t)]
            ct = cols_t[t]
            buf = np.full(P * ct, -1, np.int64)
            buf[: len(sel)] = sel
            pi[c, :, base_col[t]:base_col[t] + ct] = buf.reshape(P, ct)

    inv_pi = np.full(N, -1, np.int64)
    src_core = pi.reshape(NC, -1)
    for c in range(NC):
        flat = src_core[c]
        m = flat >= 0
        inv_pi[flat[m]] = c * (P * COLS) + np.nonzero(m)[0]
    assert (inv_pi >= 0).all()

    # per-core idx arrays [P, S] int32 (pi-space source rows, ZROW pads)
    idx = np.full((NC, P, S), ZROW, np.int32)
    for c in range(NC):
        for t in tiers:
            sel = nodes_by[(c, t)]
            ct = cols_t[t]
            n = len(sel)
            if n == 0:
                continue
            lens = deg[sel]
            tot = int(lens.sum())
            rep = np.repeat(np.arange(n), lens)
            pos = np.arange(tot) - np.repeat(np.cumsum(lens) - lens, lens)
            src_pos = np.repeat(starts[sel], lens) + pos
            mat = np.full((P * ct, t), ZROW, np.int64)
            mat[rep, pos] = inv_pi[row_s[src_pos]]
            idx[c, :, base_slot[t]:base_slot[t] + ct * t] = (
                mat.reshape(P, ct * t))

    L.N, L.E, L.NPC = N, E, NPC
    L.tiers = [(t, cols_t[t], base_slot[t], base_col[t]) for t in tiers]
    L.COLS, L.S, L.N_g, L.ZROW = COLS, S, N_g, ZROW
    L.NIT = S // P
    L.CH = NC * COLS + 1                 # call-buffer columns (see finale)
    L.pi, L.inv_pi, L.idx = pi, inv_pi, idx
    L.deg = deg
    return L


# ---------------------------------------------------------------- bass kernel

def build_kernel(L):
    COLS, S, N_g, ZROW, CH, NIT = L.COLS, L.S, L.N_g, L.ZROW, L.CH, L.NIT
    NCALL = NC * (P * COLS + F)
    dt = mybir.dt

    nc = bacc.Bacc("TRN2", target_bir_lowering=False, debug=False,
                   num_devices=NC)

    # ---- I/O
    idx_d = nc.dram_tensor("idx", [P, S], dt.int32, kind="ExternalInput")
    xg_d = nc.dram_tensor("xg", [N_g, FIN], dt.float32, kind="ExternalInput")
    xl_d = nc.dram_tensor("xl", [P, COLS * FIN], dt.float32, kind="ExternalInput")
    cho_d = nc.dram_tensor("cho", [P, COLS], dt.float32, kind="ExternalInput")
    val_d = nc.dram_tensor("valid", [P, COLS], dt.float32, kind="ExternalInput")
    stm_d = nc.dram_tensor("statm", [P, CH], dt.float32, kind="ExternalInput")
    w1_d = nc.dram_tensor("w1s", [P, P], dt.float32, kind="ExternalInput")
    w2_d = nc.dram_tensor("w2s", [P, P], dt.float32, kind="ExternalInput")
    w3_d = nc.dram_tensor("w3s", [P, 8], dt.float32, kind="ExternalInput")
    b1_d = nc.dram_tensor("bg1", [P, P], dt.float32, kind="ExternalInput")
    b2_d = nc.dram_tensor("bg2", [P, P], dt.float32, kind="ExternalInput")
    b3_d = nc.dram_tensor("b3", [1, 1], dt.float32, kind="ExternalInput")
    fcw_d = nc.dram_tensor("fcw", [F, 1], dt.float32, kind="ExternalInput")
    fcb_d = nc.dram_tensor("fcb", [1, 1], dt.float32, kind="ExternalInput")
    out_c = nc.dram_tensor("choice_out", [P, COLS], dt.float32,
                           kind="ExternalOutput")
    out_v = nc.dram_tensor("value_out", [1, 1], dt.float32,
                           kind="ExternalOutput")

    # ---- DRAM scratch
    tau1 = nc.dram_tensor("tau1", [N_g + 1, FIN], dt.float32)
    tau2 = nc.dram_tensor("tau2", [N_g + 1, F], dt.float32)
    tau3 = nc.dram_tensor("tau3", [N_g + 1, 1], dt.float32)
    grid = nc.dram_tensor("grid", [P, S * F], dt.float32)
    ccd_i = nc.dram_tensor("ccd_i", [P * COLS], dt.float32)
    ccd_o = nc.dram_tensor("ccd_o", [N_g], dt.float32)
    cch_i = nc.dram_tensor("cch_i", [P * COLS * F], dt.float32)
    ccz_i = nc.dram_tensor("ccz_i", [P * COLS], dt.float32)
    ccf_i = nc.dram_tensor("ccf_i", [P * COLS + F], dt.float32)
    ccf_o = nc.dram_tensor("ccf_o", [NCALL], dt.float32)

    RG = [list(range(NC))]

    from concourse.masks import make_identity

    with tile.TileContext(nc) as tc:
        with (
            tc.tile_pool(name="persist", bufs=1) as sp,
            tc.tile_pool(name="slab", bufs=2) as slp,
            tc.tile_pool(name="gath", bufs=2) as gp,
            tc.tile_pool(name="ixp", bufs=2) as ixp,
            tc.tile_pool(name="psum", bufs=2, space="PSUM") as pp,
            tc.tile_pool(name="psum2", bufs=2, space="PSUM") as pp2,
        ):
            # ------------------------------------------------ persistent tiles
            h1 = sp.tile([P, COLS * F], dt.float32)
            h2 = sp.tile([P, COLS * F], dt.float32)
            u16 = sp.tile([P, COLS * F], dt.float32)     # u / v scratch
            vtmp = sp.tile([P, COLS * F], dt.float32)
            xloc = sp.tile([P, COLS * FIN], dt.float32)
            dinv = sp.tile([P, COLS], dt.float32)
            snorm = sp.tile([P, COLS], dt.float32)
            z3 = sp.tile([P, COLS], dt.float32)
            u3 = sp.tile([P, COLS], dt.float32)
            cvec = sp.tile([P, COLS], dt.float32)
            chot = sp.tile([P, COLS], dt.float32)
            validt = sp.tile([P, COLS], dt.float32)
            statm = sp.tile([P, CH], dt.float32)
            callt = sp.tile([P, CH], dt.float32)
            ident = sp.tile([P, P], dt.float32)
            w1s = sp.tile([P, P], dt.float32)
            w2s = sp.tile([P, P], dt.float32)
            w3s = sp.tile([P, 8], dt.float32)
            bg1 = sp.tile([P, P], dt.float32)
            bg2 = sp.tile([P, P], dt.float32)
            nc.sync.dma_start(out=w1s[:], in_=w1_d[:, :])
            nc.sync.dma_start(out=w2s[:], in_=w2_d[:, :])
            nc.sync.dma_start(out=w3s[:], in_=w3_d[:, :])
            nc.sync.dma_start(out=bg1[:], in_=b1_d[:, :])
            nc.sync.dma_start(out=bg2[:], in_=b2_d[:, :])
            ones_c = sp.tile([P, 1], dt.float32)
            ones_r = sp.tile([1, P], dt.float32)
            zrow_t = sp.tile([1, F], dt.float32)
            w3t = sp.tile([F, 1], dt.float32, name="w3t")
            small = sp.tile([P, F], dt.float32, name="small")

            make_identity(nc, ident[:])
            nc.vector.memset(ones_c[:], 1.0)
            nc.vector.memset(ones_r[:], 1.0)
            nc.vector.memset(zrow_t[:], 0.0)

            nc.sync.dma_start(out=xloc[:], in_=xl_d[:, :])
            nc.sync.dma_start(out=chot[:], in_=cho_d[:, :])
            nc.sync.dma_start(out=validt[:], in_=val_d[:, :])
            nc.sync.dma_start(out=statm[:], in_=stm_d[:, :])

            # ------------------------------------------------ degree pass
            # deg from idx slots (slot != ZROW), tier tree sums
            def tier_chunks(fwidth):
                """Yield (slot_off, col_off, K, ccols) chunks, ccols*K*fwidth<=8192."""
                for (K, ct, sb, cb) in L.tiers:
                    step = max(1, 4096 // (K * fwidth))
                    c0 = 0
                    while c0 < ct:
                        cc = min(step, ct - c0)
                        yield (sb + c0 * K, cb + c0, K, cc)
                        c0 += cc

            def tree_reduce(buf, w, blk):
                """In-place: sum `w` blocks of `blk` elems (per 3d-view) to one."""
                while w > 1:
                    h = (w + 1) // 2
                    rem = w - h
                    nc.vector.tensor_add(
                        out=buf[:, :rem * blk],
                        in0=buf[:, :rem * blk],
                        in1=buf[:, h * blk:w * blk])
                    w = h

            for (so, co, K, cc) in tier_chunks(1):
                it_i = slp.tile([P, 4096], dt.int32, name="degi", tag="slab")
                it_f = slp.tile([P, 4096], dt.float32, name="degf", tag="slab")
                nc.sync.dma_start(out=it_i[:, :cc * K],
                                  in_=idx_d[:, so:so + cc * K])
                nc.vector.tensor_scalar(
                    out=it_f[:, :cc * K], in0=it_i[:, :cc * K],
                    scalar1=float(ZROW), scalar2=None,
                    op0=mybir.AluOpType.not_equal)
                # reduce K slots per dest: view [P, cc, K] -> sum over K
                nc.vector.tensor_reduce(
                    out=dinv[:, co:co + cc],
                    in_=it_f[:, :cc * K].rearrange("p (c k) -> p c k", k=K),
                    axis=mybir.AxisListType.X, op=mybir.AluOpType.add)

            # dinv = 1/sqrt(deg+2); snorm = 2*dinv^2
            two_t = sp.tile([P, 1], dt.float32, name="two_t")
            nc.vector.memset(two_t[:], 2.0)
            nc.scalar.activation(out=dinv[:], in_=dinv[:],
                                 func=mybir.ActivationFunctionType.Sqrt,
                                 bias=two_t[:, 0:1], scale=1.0)
            nc.vector.reciprocal(out=dinv[:], in_=dinv[:])
            nc.vector.tensor_mul(out=snorm[:], in0=dinv[:], in1=dinv[:])
            nc.vector.tensor_scalar_mul(snorm[:], snorm[:], 2.0)

            # AllGather dinv
            nc.sync.dma_start(out=ccd_i[:], in_=dinv[:])
            nc.gpsimd.collective_compute(
                "AllGather", mybir.AluOpType.bypass, replica_groups=RG,
                ins=[ccd_i[:].opt()], outs=[ccd_o[:].opt()])

            # ------------------------------------------------ tau1 = dinv * x
            ccd_v = ccd_o[:].rearrange("(p c) -> p c", p=P)      # [P, 8*COLS]
            xg_v = xg_d[:].rearrange("(p c) f -> p c f", p=P)    # [P, 8*COLS, 3]
            GC = N_g // P                                        # 8*COLS
            QS = GC // 8
            for q in range(8):
                xt = slp.tile([P, QS * FIN], dt.float32, name="xs", tag="xs")
                dt_ = slp.tile([P, QS], dt.float32, name="dgs", tag="dgs")
                nc.sync.dma_start(
                    out=xt[:].rearrange("p (c f) -> p c f", f=FIN),
                    in_=xg_v[:, q * QS:(q + 1) * QS, :])
                nc.sync.dma_start(out=dt_[:], in_=ccd_v[:, q * QS:(q + 1) * QS])
                nc.vector.tensor_mul(
                    out=xt[:].rearrange("p (c f) -> p c f", f=FIN),
                    in0=xt[:].rearrange("p (c f) -> p c f", f=FIN),
                    in1=dt_[:].to_broadcast([P, QS, FIN]))
                nc.sync.dma_start(
                    out=tau1[0:N_g, :].rearrange("(p c) f -> p c f", p=P)
                        [:, q * QS:(q + 1) * QS, :],
                    in_=xt[:].rearrange("p (c f) -> p c f", f=FIN))
            nc.sync.dma_start(out=tau1[N_g:N_g + 1, :], in_=zrow_t[:, :FIN])
            nc.sync.dma_start(out=tau2[N_g:N_g + 1, :], in_=zrow_t[:, :F])
            nc.sync.dma_start(out=tau3[N_g:N_g + 1, :], in_=zrow_t[:, :1])

            # ------------------------------------------------ gather loop maker
            def gather_layer(tau, fwidth):
                def body(it):
                    ixt = ixp.tile([P, P], dt.int32, name="ixt", tag="ixt")
                    nc.sync.dma_start(out=ixt[:],
                                      in_=idx_d[:, bass.ds(it * P, P)])
                    gt = gp.tile([P, P * fwidth], dt.float32,
                                 name="gt", tag="gt")
                    for j in range(P):
                        nc.gpsimd.indirect_dma_start(
                            out=gt[:, j * fwidth:(j + 1) * fwidth],
                            out_offset=None,
                            in_=tau[:, :],
                            in_offset=bass.IndirectOffsetOnAxis(
                                ap=ixt[:, j:j + 1], axis=0))
                    nc.sync.dma_start(
                        out=grid[:, bass.ds(it * P * fwidth, P * fwidth)],
                        in_=gt[:])
                with tc.For_i(0, NIT, 1) as it:
                    body(it)

            # tree phase: grid -> dst (dst free layout: per dest `fo` floats at
            # column stride `fstride`, offset co*fstride)
            def tree_layer(dst, fwidth, fstride):
                for (so, co, K, cc) in tier_chunks(fwidth):
                    tt = slp.tile([P, 4096], dt.float32, name="tt", tag="slab")
                    nc.sync.dma_start(
                        out=tt[:, :cc * K * fwidth],
                        in_=grid[:, so * fwidth:(so + cc * K) * fwidth])
                    # sum K blocks of fwidth for each of cc dests:
                    # view [P, cc, K*fwidth]; reduce K via strided halving on
                    # the flat [cc*K*fwidth] with block = fwidth, per dest.
                    w = K
                    while w > 1:
                        h = (w + 1) // 2
                        rem = w - h
                        nc.vector.tensor_add(
                            out=tt[:, :cc * K * fwidth]
                                .rearrange("p (c k) -> p c k", c=cc)
                                [:, :, :rem * fwidth],
                            in0=tt[:, :cc * K * fwidth]
                                .rearrange("p (c k) -> p c k", c=cc)
                                [:, :, :rem * fwidth],
                            in1=tt[:, :cc * K * fwidth]
                                .rearrange("p (c k) -> p c k", c=cc)
                                [:, :, h * fwidth:w * fwidth])
                        w = h
                    nc.vector.tensor_copy(
                        out=dst[:, co * fstride:(co + cc) * fstride]
                            .rearrange("p (c f) -> p c f", f=fstride)
                            [:, :, :fwidth],
                        in_=tt[:, :cc * K * fwidth]
                            .rearrange("p (c k) -> p c k", c=cc)
                            [:, :, :fwidth])

            # pack-matmul: h_out[:, pk*8*fo ...] = act(v^T . Wstack + bgrid)
            def packs(v, wstack, fout, dst, bgrid, relu):
                for pk in range(COLS // 8):
                    tp = pp.tile([P, P], dt.float32, name="tp", tag="tp",
                                 space="PSUM")
                    nc.tensor.transpose(out=tp[:], in_=v[:, pk * P:(pk + 1) * P],
                                        identity=ident[:])
                    lh = slp.tile([P, P], dt.float32, name="lh", tag="lh")
                    nc.vector.tensor_copy(out=lh[:], in_=tp[:])
                    op = pp2.tile([P, 8 * fout], dt.float32, name="op", tag="op",
                                  space="PSUM")
                    nc.tensor.matmul(out=op[:], lhsT=lh[:], rhs=wstack[:, :8 * fout],
                                     start=True, stop=True)
                    seg = dst[:, pk * 8 * fout:(pk + 1) * 8 * fout]
                    if bgrid is not None:
                        nc.vector.tensor_add(out=seg, in0=op[:],
                                             in1=bgrid[:, :8 * fout])
                    else:
                        nc.vector.tensor_copy(out=seg, in_=op[:])
                    if relu:
                        nc.scalar.activation(
                            out=seg, in_=seg,
                            func=mybir.ActivationFunctionType.Relu)

            db3 = dinv[:].to_broadcast([P, COLS, FIN])
            db16 = dinv[:].to_broadcast([P, COLS, F])
            sb3 = snorm[:].to_broadcast([P, COLS, FIN])
            sb16 = snorm[:].to_broadcast([P, COLS, F])

            def v3(t):
                return t.rearrange("p (c f) -> p c f", f=FIN)

            def v16(t):
                return t.rearrange("p (c f) -> p c f", f=F)

            # ------------------------------------------------ layer 1
            gather_layer(tau1, FIN)
            nc.vector.memset(u16[:], 0.0)
            tree_layer(u16, FIN, F)          # u in 0:3 of each 16-block
            # v1 = dinv*u + snorm*x_loc   (into u16's 0:3 slices)
            nc.vector.tensor_mul(out=v16(u16[:])[:, :, :FIN],
                                 in0=v16(u16[:])[:, :, :FIN], in1=db3)
            nc.vector.tensor_mul(out=v3(xloc[:]), in0=v3(xloc[:]), in1=sb3)
            nc.vector.tensor_add(out=v16(u16[:])[:, :, :FIN],
                                 in0=v16(u16[:])[:, :, :FIN], in1=v3(xloc[:]))
            packs(u16, w1s, F, h1, bg1, relu=True)

            # tau2 = allgather(dinv * h1)
            nc.vector.tensor_mul(out=v16(vtmp[:]), in0=v16(h1[:]), in1=db16)
            nc.sync.dma_start(out=cch_i[:], in_=vtmp[:])
            nc.gpsimd.collective_compute(
                "AllGather", mybir.AluOpType.bypass, replica_groups=RG,
                ins=[cch_i[:].opt()],
                outs=[tau2[0:N_g, :].opt()])

            # ------------------------------------------------ layer 2
            gather_layer(tau2, F)
            tree_layer(u16, F, F)
            # v2 = dinv*u + snorm*h1
            nc.vector.tensor_mul(out=v16(u16[:]), in0=v16(u16[:]), in1=db16)
            nc.vector.tensor_mul(out=v16(vtmp[:]), in0=v16(h1[:]), in1=sb16)
            nc.vector.tensor_add(out=u16[:], in0=u16[:], in1=vtmp[:])
            packs(u16, w2s, F, h2, bg2, relu=True)

            # z3 = h2 @ W3
            packs(h2, w3s, 1, z3, None, relu=False)

            # tau3 = allgather(dinv * z3)
            nc.vector.tensor_mul(out=cvec[:], in0=z3[:], in1=dinv[:])
            nc.sync.dma_start(out=ccz_i[:], in_=cvec[:])
            nc.gpsimd.collective_compute(
                "AllGather", mybir.AluOpType.bypass, replica_groups=RG,
                ins=[ccz_i[:].opt()],
                outs=[tau3[0:N_g, :].opt()])

            # ------------------------------------------------ layer 3
            gather_layer(tau3, 1)
            tree_layer(u3, 1, 1)
            # c = dinv*u3 + snorm*z3 + b3
            nc.vector.tensor_mul(out=u3[:], in0=u3[:], in1=dinv[:])
            nc.vector.tensor_mul(out=cvec[:], in0=z3[:], in1=snorm[:])
            nc.vector.tensor_add(out=cvec[:], in0=cvec[:], in1=u3[:])
            b3t = sp.tile([1, 1], dt.float32, name="b3t")
            nc.sync.dma_start(out=b3t[:], in_=b3_d[:, :])
            b3b_p = pp.tile([P, 1], dt.float32, name="b3b", tag="bcast",
                            space="PSUM")
            nc.tensor.matmul(out=b3b_p[:], lhsT=ones_r[:], rhs=b3t[:],
                             start=True, stop=True)
            b3b = sp.tile([P, 1], dt.float32, name="b3b_s")
            nc.vector.tensor_copy(out=b3b[:], in_=b3b_p[:])
            nc.vector.tensor_scalar(out=cvec[:], in0=cvec[:], scalar1=b3b[:, 0:1],
                                    scalar2=None, op0=mybir.AluOpType.add)

            # ------------------------------------------------ finale
            # logits = c*choices + (choices-1)*1e30   (exact for chosen)
            lg = u3                                    # reuse
            mtmp = sp.tile([P, COLS], dt.float32, name="mtmp")
            nc.vector.tensor_scalar(out=mtmp[:], in0=chot[:], scalar1=1.0,
                                    scalar2=1e30,
                                    op0=mybir.AluOpType.subtract,
                                    op1=mybir.AluOpType.mult)
            nc.vector.tensor_mul(out=lg[:], in0=cvec[:], in1=chot[:])
            nc.vector.tensor_add(out=lg[:], in0=lg[:], in1=mtmp[:])

            # masked h2 sum -> [16,1]
            nc.vector.tensor_mul(out=v16(vtmp[:]), in0=v16(h2[:]),
                                 in1=validt[:].to_broadcast([P, COLS, F]))
            hs = sp.tile([P, F], dt.float32, name="hs")
            nc.vector.tensor_reduce(
                out=hs[:],
                in_=vtmp[:].rearrange("p (c f) -> p f c", f=F),
                axis=mybir.AxisListType.X, op=mybir.AluOpType.add)
            hsp = pp.tile([F, 1], dt.float32, name="hsp", tag="bcast",
                          space="PSUM")
            nc.tensor.matmul(out=hsp[:], lhsT=hs[:], rhs=ones_c[:],
                             start=True, stop=True)
            hsv = sp.tile([F, 1], dt.float32, name="hsv")
            nc.vector.tensor_copy(out=hsv[:], in_=hsp[:])

            # call buffer = [logits | h2sum] allgathered
            nc.sync.dma_start(out=ccf_i[0:P * COLS], in_=lg[:])
            nc.sync.dma_start(out=ccf_i[P * COLS:P * COLS + F], in_=hsv[:])
            nc.gpsimd.collective_compute(
                "AllGather", mybir.AluOpType.bypass, replica_groups=RG,
                ins=[ccf_i[:].opt()], outs=[ccf_o[:].opt()])
            nc.sync.dma_start(out=callt[:],
                              in_=ccf_o[:].rearrange("(p c) -> p c", p=P))

            # stats input: mask out the h2sum slots (exact form)
            st = sp.tile([P, CH], dt.float32, name="st")
            sttmp = sp.tile([P, CH], dt.float32, name="sttmp")
            nc.vector.tensor_scalar(out=sttmp[:], in0=statm[:], scalar1=1.0,
                                    scalar2=1e30,
                                    op0=mybir.AluOpType.subtract,
                                    op1=mybir.AluOpType.mult)
            nc.vector.tensor_mul(out=st[:], in0=callt[:], in1=statm[:])
            nc.vector.tensor_add(out=st[:], in0=st[:], in1=sttmp[:])

            rmax = sp.tile([P, 1], dt.float32, name="rmax")
            nc.vector.tensor_reduce(out=rmax[:], in_=st[:],
                                    axis=mybir.AxisListType.X,
                                    op=mybir.AluOpType.max)
            rmp = pp.tile([1, P], dt.float32, name="rmp", tag="bcast",
                          space="PSUM")
            nc.tensor.transpose(out=rmp[:], in_=rmax[:], identity=ident[:])
            rms = sp.tile([1, P], dt.float32, name="rms")
            nc.vector.tensor_copy(out=rms[:], in_=rmp[:])
            gmax = sp.tile([1, 1], dt.float32, name="gmax")
            nc.vector.tensor_reduce(out=gmax[:], in_=rms[:],
                                    axis=mybir.AxisListType.X,
                                    op=mybir.AluOpType.max)
            nc.vector.tensor_scalar_mul(gmax[:], gmax[:], -1.0)
            ngb_p = pp.tile([P, 1], dt.float32, name="ngb", tag="bcast",
                            space="PSUM")
            nc.tensor.matmul(out=ngb_p[:], lhsT=ones_r[:], rhs=gmax[:],
                             start=True, stop=True)
            ngb = sp.tile([P, 1], dt.float32, name="ngb_s")
            nc.vector.tensor_copy(out=ngb[:], in_=ngb_p[:])

            # e = exp(st - gmax), row sums
            rsum = sp.tile([P, 1], dt.float32, name="rsum")
            nc.scalar.activation(out=st[:], in_=st[:],
                                 func=mybir.ActivationFunctionType.Exp,
                                 bias=ngb[:, 0:1], scale=1.0,
                                 accum_out=rsum[:, 0:1])
            gsp = pp.tile([1, 1], dt.float32, name="gsp", tag="bcast",
                          space="PSUM")
            nc.tensor.matmul(out=gsp[:], lhsT=rsum[:], rhs=ones_c[:],
                             start=True, stop=True)
            gsum = sp.tile([1, 1], dt.float32, name="gsum")
            nc.vector.tensor_copy(out=gsum[:], in_=gsp[:])
            nc.vector.reciprocal(out=gsum[:], in_=gsum[:])
            ivp = pp.tile([P, 1], dt.float32, name="ivp", tag="bcast",
                          space="PSUM")
            nc.tensor.matmul(out=ivp[:], lhsT=ones_r[:], rhs=gsum[:],
                             start=True, stop=True)
            ivb = sp.tile([P, 1], dt.float32, name="ivb")
            nc.vector.tensor_copy(out=ivb[:], in_=ivp[:])

            # choice = exp(logits - gmax) * inv_sum
            nc.scalar.activation(out=lg[:], in_=lg[:],
                                 func=mybir.ActivationFunctionType.Exp,
                                 bias=ngb[:, 0:1], scale=1.0)
            nc.vector.tensor_scalar(out=lg[:], in0=lg[:], scalar1=ivb[:, 0:1],
                                    scalar2=None, op0=mybir.AluOpType.mult)
            nc.sync.dma_start(out=out_c[:, :], in_=lg[:])

            # value = (sum over cores of h2sum) . fc_w / N + fc_b
            vs = sp.tile([NC, F], dt.float32, name="vs")
            nc.sync.dma_start(
                out=vs[:],
                in_=ccf_o[:].rearrange("(k c) -> k c", k=NC)[:, P * COLS:P * COLS + F])
            vsp = pp.tile([F, 1], dt.float32, name="vsp", tag="bcast",
                          space="PSUM")
            nc.tensor.matmul(out=vsp[:], lhsT=vs[:], rhs=ones_c[0:NC, :],
                             start=True, stop=True)
            sv = sp.tile([F, 1], dt.float32, name="sv")
            nc.vector.tensor_copy(out=sv[:], in_=vsp[:])
            nc.sync.dma_start(out=w3t[:], in_=fcw_d[:, :])
            nc.vector.tensor_mul(out=sv[:], in0=sv[:], in1=w3t[:])
            vvp = pp.tile([1, 1], dt.float32, name="vvp", tag="bcast",
                          space="PSUM")
            nc.tensor.matmul(out=vvp[:], lhsT=sv[:], rhs=ones_c[0:F, :],
                             start=True, stop=True)
            vv = sp.tile([1, 1], dt.float32, name="vv")
            nc.vector.tensor_copy(out=vv[:], in_=vvp[:])
            nc.vector.tensor_scalar_mul(vv[:], vv[:], 1.0 / L.N)
            fcbt = sp.tile([1, 1], dt.float32, name="fcbt")
            nc.sync.dma_start(out=fcbt[:], in_=fcb_d[:, :])
            nc.vector.tensor_add(out=vv[:], in0=vv[:], in1=fcbt[:])
            nc.sync.dma_start(out=out_v[:, :], in_=vv[:])

    nc.compile()
    return nc


# ---------------------------------------------------------------- entry point

def _prep_inputs(L, x, choices, W1, b1, W2, b2, W3, b3, fc_w, fc_b):
    COLS, N_g, CH = L.COLS, L.N_g, L.CH
    x = np.asarray(x, np.float32)

    xg = np.zeros((N_g, FIN), np.float32)
    flat_pi = L.pi.reshape(-1)
    m = flat_pi >= 0
    xg[m] = x[flat_pi[m]]

    def blockdiag(w, fout):
        st = np.zeros((P, 8 * fout), np.float32)
        for t in range(8):
            st[16 * t:16 * t + w.shape[0], fout * t:fout * (t + 1)] = w
        return st

    w1s = blockdiag(np.asarray(W1, np.float32), F)
    w2s = blockdiag(np.asarray(W2, np.float32), F)
    w3s = blockdiag(np.asarray(W3, np.float32).reshape(F, 1), 1)
    bg1 = np.tile(np.asarray(b1, np.float32).reshape(1, F), (P, 8))
    bg2 = np.tile(np.asarray(b2, np.float32).reshape(1, F), (P, 8))
    ch_f = np.asarray(choices, np.float32)

    stat_flat = np.zeros(NC * (P * COLS + F), np.float32)
    for k in range(NC):
        stat_flat[k * (P * COLS + F): k * (P * COLS + F) + P * COLS] = 1.0
    statm = stat_flat.reshape(P, CH)

    in_maps = []
    for c in range(NC):
        pic = L.pi[c]                           # [P, COLS]
        mloc = pic >= 0
        xl = np.zeros((P, COLS, FIN), np.float32)
        xl[mloc] = x[pic[mloc]]
        cho = np.zeros((P, COLS), np.float32)
        cho[mloc] = ch_f[pic[mloc]]
        valid = mloc.astype(np.float32)
        in_maps.append(dict(
            idx=L.idx[c],
            xg=xg,
            xl=xl.reshape(P, COLS * FIN),
            cho=cho,
            valid=valid,
            statm=statm,
            w1s=w1s, w2s=w2s, w3s=w3s, bg1=bg1, bg2=bg2,
            b3=np.asarray(b3, np.float32).reshape(1, 1),
            fcw=np.asarray(fc_w, np.float32).reshape(F, 1),
            fcb=np.asarray(fc_b, np.float32).reshape(1, 1),
        ))
    return in_maps


def run(L, nc, in_maps):
    res = bass_utils.run_bass_kernel_spmd(nc, in_maps, core_ids=list(range(NC)))
    choice = np.zeros(L.N, np.float32)
    for c in range(NC):
        pic = L.pi[c]
        m = pic >= 0
        choice[pic[m]] = res.results[c]["choice_out"][m]
    value = res.results[0]["value_out"].reshape(1, 1)
    return choice, value


_CACHE = {}


def kernel(x, edge_index, choices, W1, b1, W2, b2, W3, b3, fc_w, fc_b):
    x = np.asarray(x)
    N = x.shape[0]
    key = ("k", N, np.asarray(edge_index).shape[1])
    if key not in _CACHE:
        L = build_layout(np.asarray(edge_index), N)
        nc = build_kernel(L)
        _CACHE[key] = (L, nc)
    L, nc = _CACHE[key]
    in_maps = _prep_inputs(L, x, choices, W1, b1, W2, b2, W3, b3, fc_w, fc_b)
    return run(L, nc, in_maps)
